# revision 1
# baseline (speedup 1.0000x reference)
"""LocalFrameAttention TRN2 kernel.

Problem: x[B=2,F=16,N=256,D=1024] -> qkv proj -> chunked local attention
(chunk = 4 frames = 1024 tokens; chunk c attends to chunks {c-1, c}, chunk 0
to itself) -> out proj.  H=16 heads, HD=64.

Sharding: 8 cores = B(2) x head-groups(4).  Each core handles 4 heads for all
16 frames of one batch: column-parallel qkv projection, full SDPA for its
heads, row-parallel out projection producing a partial [D, T] output; the
host sums the 4 partials per batch (tensor-parallel reduce) and transposes.

Layouts (on-chip activations kept "transposed", d-major):
  - XT [D, T] host-transposed; K^T/Q^T [e=256, T] via lhsT=W^T, rhs=XT;
  - V [T, e] via lhsT=XT, rhs=Wv^T, stored [128, tok_tile, head, 65] with a
    ones 65th column (softmax denominator via the PV matmul);
  - S^T [k_tok=128, q 512 x 2 heads] = mm(lhsT=K^T, rhs=Q^T), K=64
    contraction, two heads row-paired on the PE;
  - P^T = exp(S^T) on ACT (no max subtraction; |logits| small), fp16;
  - O^T accum [65, 512] = mm(lhsT=[V|1], rhs=P^T) over window k-tiles;
  - normalize: O^T copied out of PSUM immediately (frees accumulator banks),
    reciprocal of row 64, partition-broadcast via a DRAM round-trip DMA,
    DVE muls;
  - out proj: partial^T [dd, tok] = mm(lhsT=Wo^T, rhs=O^T).

All matmul operands are fp16 (fp32 PSUM accumulation).  fp16 gets separate,
pipelined LDWEIGHTS (4-byte dtypes force a self-loading matmul that
serializes the weight load) at 2x the mantissa error of fp32r (2^-11).
Next-chunk projection and prev-chunk out-projection matmul groups are
interleaved into the SDPA loop as stall filler (engine instruction order is
static on TRN2).

Measured: ~455 us/core for the full pass (all 8 cores run in parallel),
max relative error vs the fp32 reference ~5e-4.
"""

import sys

if "/opt/trn_rl_repo" not in sys.path:
    sys.path.insert(0, "/opt/trn_rl_repo")

import numpy as np

import concourse.bass as bass  # noqa: F401
import concourse.mybir as mybir
import concourse.tile as tile
from concourse import bacc
from concourse.bass_utils import run_bass_kernel_spmd

F32 = mybir.dt.float32
F32R = mybir.dt.float32r
F16 = mybir.dt.float16
EXP = mybir.ActivationFunctionType.Exp

B, F, N, D = 2, 16, 256, 1024
H, HD, CHUNK = 16, 64, 4
C = F // CHUNK            # 4 chunks
CT = CHUNK * N            # 1024 tokens per chunk
T = F * N                 # 4096 tokens per batch
HL = 4                    # heads per core
E = HL * HD               # 256 local qkv width
NCORES = 8

_cached = {}


def _round_fp32r(a: np.ndarray) -> np.ndarray:
    """Round fp32 array to fp32r (11-bit mantissa, value in high 20 bits)."""
    u = np.ascontiguousarray(a, dtype=np.float32).view(np.uint32)
    r = ((u.astype(np.uint64) + 0x800) & 0xFFFFF000).astype(np.uint32)
    return r.view(np.float32)


def _emit_pass(nc, pools, aps, stage):
    """One full compute pass, software-pipelined: next-chunk projection and
    previous-chunk out-projection matmul groups are interleaved into the SDPA
    kt loop so the PE's static instruction order has independent filler work
    at every exp-dependency stall point."""
    (xpool, kpool, qpool, vpool, ppool, opool, spool, fpool, dpool,
     ps_s, ps_o, ps_m) = pools
    xt_d, pt_d, wkq_sb, wv_sb, wo_sb, ones_sb = aps

    kt_ring = {}
    qt_ring = {}
    v_ring = {}
    xt_tiles = {}
    ot_ring = {}

    def proj_closures(c):
        """19 closures: tile allocs + per-token-block DMA + 8 matmul groups."""
        cl = []

        def alloc(c=c):
            kt_ring[c] = kpool.tile([128, 2, CT], F16, tag="kt", name=f"kt{c}")
            qt_ring[c] = qpool.tile([128, 2, CT], F16, tag="qt", name=f"qt{c}")
            v_c = vpool.tile([128, 8, HL, 68], F16, tag="v")
            nc.vector.memset(v_c[:], 1.0)  # ones col; rest overwritten
            v_ring[c] = v_c

        cl.append(alloc)
        for tb in range(2):
            def dma(c=c, tb=tb):
                xt_t = xpool.tile([128, 8, 512], F16, tag="xt")
                t0 = c * CT + tb * 512
                nc.sync.dma_start(
                    xt_t[:],
                    xt_d[:, t0 : t0 + 512].rearrange("(dt p) t -> p dt t", p=128),
                )
                xt_tiles[(c, tb)] = xt_t

            cl.append(dma)
            for et in range(4):  # K^T (0,1), Q^T (2,3)
                def kq_group(c=c, tb=tb, et=et):
                    xt_t = xt_tiles[(c, tb)]
                    ps = ps_m.tile([128, 512], F32, tag="m")
                    for dt in range(8):
                        nc.tensor.matmul(
                            ps[:],
                            wkq_sb[:, dt, et * 128 : (et + 1) * 128],
                            xt_t[:, dt, :],
                            start=(dt == 0),
                            stop=(dt == 7),
                        )
                    dst = kt_ring[c] if et < 2 else qt_ring[c]
                    nc.vector.tensor_copy(
                        dst[:, et % 2, tb * 512 : (tb + 1) * 512], ps[:]
                    )

                cl.append(kq_group)
            for tt in range(4):  # V tok-tiles
                def v_group(c=c, tb=tb, tt=tt):
                    xt_t = xt_tiles[(c, tb)]
                    ps = ps_m.tile([128, 512], F32, tag="m")
                    for dt in range(8):
                        nc.tensor.matmul(
                            ps[:, 0:E],
                            xt_t[:, dt, tt * 128 : (tt + 1) * 128],
                            wv_sb[:, dt, :],
                            start=(dt == 0),
                            stop=(dt == 7),
                        )
                    nc.vector.tensor_copy(
                        v_ring[c][:, tb * 4 + tt, :, 0:64],
                        ps[:, 0:E].rearrange("p (h d) -> p h d", h=HL),
                    )

                cl.append(v_group)
        return cl

    def outproj_closures(c):
        cl = []
        for ddt in range(8):
            for tb in range(2):
                def op_group(c=c, ddt=ddt, tb=tb):
                    ot_c = ot_ring[c]
                    fp = ps_m.tile([128, 512], F32, tag="m")
                    for et in range(2):
                        nc.tensor.matmul(
                            fp[:],
                            wo_sb[:, et, ddt * 128 : (ddt + 1) * 128],
                            ot_c[:, et, tb * 512 : (tb + 1) * 512],
                            start=(et == 0),
                            stop=(et == 1),
                        )
                    fin = fpool.tile([128, 512], F32, tag="fin")
                    nc.vector.tensor_copy(fin[:], fp[:])
                    nc.sync.dma_start(
                        pt_d[
                            ddt * 128 : (ddt + 1) * 128,
                            c * CT + tb * 512 : c * CT + (tb + 1) * 512,
                        ],
                        fin[:],
                    )

                cl.append(op_group)
        return cl

    def sdpa_chunk(c, filler):
        """Emit SDPA(c); pop one filler closure after every few kt steps."""
        kts = (
            [(c, i) for i in range(8)]
            if c == 0
            else [(c - 1, i) for i in range(8)] + [(c, i) for i in range(8)]
        )
        ot_ring[c] = opool.tile([128, 2, CT], F16, tag="ot", name=f"ot{c}")
        ot_c = ot_ring[c]
        qt_c = qt_ring[c]

        n_steps = 4 * len(kts)
        stride = max(1, -(-n_steps // max(1, len(filler))) )
        step = 0

        if stage == "proj":
            nc.vector.tensor_copy(ot_c[:, :, 0:4], kt_ring[c][:, :, 0:4])
            while filler:
                filler.pop(0)()
            return

        for hp in range(2):
            for qb in range(2):
                o0 = ps_o.tile([65, 512], F32, tag="o")
                o1 = ps_o.tile([65, 512], F32, tag="o")
                last = len(kts) - 1
                for i, (kc, kt) in enumerate(kts):
                    s = ps_s.tile([128, 1024], F32, tag="s")
                    ktile = kt_ring[kc]
                    for hl in range(2):
                        r0, r1 = hl * 64, hl * 64 + 64
                        nc.tensor.matmul(
                            s[:, hl * 512 : hl * 512 + 512],
                            ktile[r0:r1, hp, kt * 128 : (kt + 1) * 128],
                            qt_c[r0:r1, hp, qb * 512 : (qb + 1) * 512],
                            start=True,
                            stop=True,
                        )
                    p = ppool.tile([128, 1024], F16, tag="p")
                    if stage in ("qkexp", "full"):
                        nc.scalar.activation(p[:], s[:], EXP)
                    else:
                        nc.vector.tensor_copy(p[:, 0:4], s[:, 0:4])
                    if stage == "full":
                        vt = v_ring[kc]
                        nc.tensor.matmul(
                            o0[:],
                            vt[:, kt, 2 * hp, 0:65],
                            p[:, 0:512],
                            start=(i == 0),
                            stop=(i == last),
                        )
                        nc.tensor.matmul(
                            o1[:],
                            vt[:, kt, 2 * hp + 1, 0:65],
                            p[:, 512:1024],
                            start=(i == 0),
                            stop=(i == last),
                        )
                    elif i == 0:
                        nc.vector.tensor_copy(o0[:, 0:4], p[0:65, 0:4])
                        nc.vector.tensor_copy(o1[:, 0:4], p[0:65, 0:4])
                    step += 1
                    if filler and step % stride == 0:
                        filler.pop(0)()
                # stage O^T out of PSUM immediately (frees the accumulator
                # banks for the next block's PV), then normalize from SBUF
                osb = spool.tile([65, 1024], F32, tag="osb")
                nc.vector.tensor_copy(osb[:, 0:512], o0[:])
                nc.vector.tensor_copy(osb[:, 512:1024], o1[:])
                rd0 = spool.tile([1, 512], F32, tag="rd0")
                rd1 = spool.tile([1, 512], F32, tag="rd1")
                nc.vector.reciprocal(rd0[:], osb[64:65, 0:512])
                nc.vector.reciprocal(rd1[:], osb[64:65, 512:1024])
                rd_dram = dpool.tile([2, 512], F32, tag="rdd")
                nc.sync.dma_start(rd_dram[0:1, :], rd0[:])
                nc.sync.dma_start(rd_dram[1:2, :], rd1[:])
                bc0 = spool.tile([64, 512], F32, tag="bc0")
                bc1 = spool.tile([64, 512], F32, tag="bc1")
                nc.sync.dma_start(bc0[:], rd_dram[0:1, :].to_broadcast((64, 512)))
                nc.sync.dma_start(bc1[:], rd_dram[1:2, :].to_broadcast((64, 512)))
                qs = slice(qb * 512, qb * 512 + 512)
                nc.vector.tensor_mul(ot_c[0:64, hp, qs], osb[0:64, 0:512], bc0[:])
                nc.vector.tensor_mul(ot_c[64:128, hp, qs], osb[0:64, 512:1024], bc1[:])
        while filler:
            filler.pop(0)()

    # prologue: chunk 0 projections
    for cl in proj_closures(0):
        cl()
    for c in range(C):
        filler = []
        pj = proj_closures(c + 1) if c + 1 < C else []
        op = outproj_closures(c - 1) if c >= 1 else []
        # interleave: out-proj groups are ready immediately; proj groups
        # depend on the xt DMA. Alternate so the PE always has ready filler.
        while pj or op:
            if op:
                filler.append(op.pop(0))
            if pj:
                filler.append(pj.pop(0))
        sdpa_chunk(c, filler)
    for cl in outproj_closures(C - 1):
        cl()


def _build(reps: int = 1, stage: str = "full", hw_loop: int = 1):
    nc = bacc.Bacc(
        "TRN2",
        target_bir_lowering=False,
        debug=False,
        enable_asserts=False,
        num_devices=NCORES,
    )
    xt_d = nc.dram_tensor("xt", [D, T], F16, kind="ExternalInput").ap()
    wkq_d = nc.dram_tensor("wkq", [D, 2 * E], F16, kind="ExternalInput").ap()
    wv_d = nc.dram_tensor("wv", [D, E], F16, kind="ExternalInput").ap()
    wo_d = nc.dram_tensor("wo", [E, D], F16, kind="ExternalInput").ap()
    ones_d = nc.dram_tensor("ones", [1, 64], F16, kind="ExternalInput").ap()
    pt_d = nc.dram_tensor("pt", [D, T], F32, kind="ExternalOutput").ap()

    with tile.TileContext(nc) as tc:
        with (
            tc.tile_pool(name="const", bufs=1) as cpool,
            tc.tile_pool(name="xt", bufs=3) as xpool,
            tc.tile_pool(name="kt", bufs=4) as kpool,
            tc.tile_pool(name="qt", bufs=2) as qpool,
            tc.tile_pool(name="v", bufs=4) as vpool,
            tc.tile_pool(name="p", bufs=8) as ppool,
            tc.tile_pool(name="ot", bufs=2) as opool,
            tc.tile_pool(name="sm", bufs=4) as spool,
            tc.tile_pool(name="fin", bufs=4) as fpool,
            tc.tile_pool(name="dram", bufs=4, space="DRAM") as dpool,
            tc.tile_pool(name="ps_s", bufs=2, space="PSUM") as ps_s,
            tc.tile_pool(name="ps_o", bufs=2, space="PSUM") as ps_o,
            tc.tile_pool(name="ps_m", bufs=2, space="PSUM") as ps_m,
        ):
            wkq_sb = cpool.tile([128, 8, 2 * E], F16, tag="wkq")
            nc.sync.dma_start(wkq_sb[:], wkq_d.rearrange("(dt p) e -> p dt e", p=128))
            wv_sb = cpool.tile([128, 8, E], F16, tag="wv")
            nc.sync.dma_start(wv_sb[:], wv_d.rearrange("(dt p) e -> p dt e", p=128))
            wo_sb = cpool.tile([128, 2, D], F16, tag="wo")
            nc.sync.dma_start(wo_sb[:], wo_d.rearrange("(et p) d -> p et d", p=128))
            ones_sb = cpool.tile([1, 64], F16, tag="ones")
            nc.sync.dma_start(ones_sb[:], ones_d)

            pools = (xpool, kpool, qpool, vpool, ppool, opool, spool,
                     fpool, dpool, ps_s, ps_o, ps_m)
            aps = (xt_d, pt_d, wkq_sb, wv_sb, wo_sb, ones_sb)

            if hw_loop > 1:
                with tc.For_i(0, hw_loop, 1):
                    _emit_pass(nc, pools, aps, stage)
            else:
                for _ in range(reps):
                    _emit_pass(nc, pools, aps, stage)

    nc.compile()
    return nc


def _prepare_inputs(x, w_qkv, w_out):
    xt = []
    for b in range(B):
        xt.append(np.ascontiguousarray(x[b].reshape(T, D).T).astype(np.float16))
    ones = np.ones((1, 64), dtype=np.float16)
    in_maps = []
    for core in range(NCORES):
        b, hg = divmod(core, HL)
        e0 = hg * E
        wq = 0.125 * w_qkv[e0 : e0 + E]                 # fold 1/sqrt(HD)
        wk = w_qkv[H * HD + e0 : H * HD + e0 + E]
        wv = w_qkv[2 * H * HD + e0 : 2 * H * HD + e0 + E]
        wkq = np.ascontiguousarray(np.concatenate([wk, wq], axis=0).T).astype(np.float16)
        wv_t = np.ascontiguousarray(wv.T).astype(np.float16)
        wo_t = np.ascontiguousarray(w_out[:, e0 : e0 + E].T).astype(np.float16)
        in_maps.append(
            {"xt": xt[b], "wkq": wkq, "wv": wv_t, "wo": wo_t, "ones": ones}
        )
    return in_maps


def _gather(results):
    out = np.empty((B, F, N, D), dtype=np.float32)
    for b in range(B):
        acc = results[4 * b]["pt"].copy()
        for hg in range(1, HL):
            acc += results[4 * b + hg]["pt"]
        out[b] = acc.T.reshape(F, N, D)
    return out


def run(x, w_qkv, w_out, trace=False, reps=1, stage="full", hw_loop=1):
    key = ("nc", reps, stage, hw_loop)
    if key not in _cached:
        _cached[key] = _build(reps, stage, hw_loop)
    nc = _cached[key]
    in_maps = _prepare_inputs(
        np.asarray(x, dtype=np.float32),
        np.asarray(w_qkv, dtype=np.float32),
        np.asarray(w_out, dtype=np.float32),
    )
    res = run_bass_kernel_spmd(nc, in_maps, core_ids=list(range(NCORES)), trace=trace)
    return _gather(res.results), res


def kernel(x, w_qkv, w_out):
    out, _ = run(x, w_qkv, w_out)
    return out



# revision 3
# speedup vs baseline: 1.1076x; 1.1076x over previous
"""LocalFrameAttention TRN2 kernel (v2).

Problem: x[B=2,F=16,N=256,D=1024] -> qkv proj -> chunked local attention
(chunk = 4 frames = 1024 tokens; chunk c attends to chunks {c-1, c}, chunk 0
to itself) -> out proj.  H=16 heads, HD=64.

Sharding: 8 cores = B(2) x head-groups(4).  Each core handles 4 heads for all
16 frames of one batch: column-parallel qkv projection, full SDPA for its
heads, row-parallel out projection producing a partial [D, T] fp16 output;
the host sums the 4 partials per batch and transposes.

The kernel is ACT(exp)-bound: 224 exp instructions on [128,1024] PSUM tiles
~= 1.15us each ~= 257us.  Everything else is arranged to hide under that:

- QK pairs row-tiled (two 64-contraction heads at array row groups 0/64 run
  concurrently), S fp32 ping-pong in 4 psum banks.
- PV pairs col-tiled (two heads' 64-row outputs at array col groups 0/64,
  concurrent) accumulating into ONE psum bank [128,512].
- softmax denominators via 4-way col-tiled ones-matmuls (M=32 strips at col
  positions 0/32/64/96, one 4-way group per kt-step pair; h0 partials land in
  strips 0+64, h1 in 32+96).
- denominator sum+broadcast fused into one col-tiled pair of selection-matrix
  matmuls (sel.T @ den -> 64 identical rows of h's denominator), then a DVE
  reciprocal + one [128,512] multiply normalizes both heads at once.  No DRAM
  round-trip broadcast (which serialized the old block epilogue).
- PSUM: S 2x2 + O 1 + den 1 + proj/misc 2 = exactly 8 banks.
- per-iteration PE emission order [fillers, QK(i+2), PV(i), dens] so the PE
  always has independent work queued ahead of the exp-dependent ops;
  normalize tails deferred into the following block as priority fillers.
- projection / out-projection matmul groups interleaved into the SDPA stream
  as stall filler (engine instruction order is static on TRN2).

All matmul operands fp16 (fp32 PSUM).  pt output fp16.
"""

import sys

if "/opt/trn_rl_repo" not in sys.path:
    sys.path.insert(0, "/opt/trn_rl_repo")

import numpy as np

import concourse.bass as bass  # noqa: F401
import concourse.mybir as mybir
import concourse.tile as tile
from concourse import bacc
from concourse.bass_utils import run_bass_kernel_spmd

F32 = mybir.dt.float32
F16 = mybir.dt.float16
EXP = mybir.ActivationFunctionType.Exp

B, F, N, D = 2, 16, 256, 1024
H, HD, CHUNK = 16, 64, 4
C = F // CHUNK            # 4 chunks
CT = CHUNK * N            # 1024 tokens per chunk
T = F * N                 # 4096 tokens per batch
HL = 4                    # heads per core
E = HL * HD               # 256 local qkv width
NCORES = 8

_cached = {}


def _emit_pass(nc, pools, aps, stage):
    (xpool, kpool, qpool, vpool, ppool, opool, spool, fpool,
     ps_s, ps_o, ps_den, ps_m) = pools
    xt_d, pt_d, wkq_sb, wv_sb, wo_sb, cst_sb = aps

    ones32 = cst_sb[:, 0:32]     # all-ones [128,32] (denominator lhsT)
    sel0 = cst_sb[:, 32:96]      # ones at rows {0,64}  -> h_even denom bcast
    sel1 = cst_sb[:, 96:160]     # ones at rows {32,96} -> h_odd denom bcast

    kt_ring, qt_ring, v_ring, xt_tiles, ot_ring = {}, {}, {}, {}, {}

    def proj_closures(c):
        """19 closures: tile allocs + per-token-block DMA + 8 matmul groups."""
        cl = []

        def alloc(c=c):
            kt_ring[c] = kpool.tile([128, 2, CT], F16, tag="kt", name=f"kt{c}")
            qt_ring[c] = qpool.tile([128, 2, CT], F16, tag="qt", name=f"qt{c}")
            v_ring[c] = vpool.tile([128, 8, HL, HD], F16, tag="v", name=f"v{c}")

        cl.append(alloc)
        for tb in range(2):
            def dma(c=c, tb=tb):
                xt_t = xpool.tile([128, 8, 512], F16, tag="xt")
                t0 = c * CT + tb * 512
                nc.sync.dma_start(
                    xt_t[:],
                    xt_d[:, t0 : t0 + 512].rearrange("(dt p) t -> p dt t", p=128),
                )
                xt_tiles[(c, tb)] = xt_t

            cl.append(dma)
            for et in range(4):  # K^T (0,1), Q^T (2,3)
                def kq_group(c=c, tb=tb, et=et):
                    xt_t = xt_tiles[(c, tb)]
                    ps = ps_m.tile([128, 512], F32, tag="m")
                    for dt in range(8):
                        nc.tensor.matmul(
                            ps[:],
                            wkq_sb[:, dt, et * 128 : (et + 1) * 128],
                            xt_t[:, dt, :],
                            start=(dt == 0),
                            stop=(dt == 7),
                        )
                    dst = kt_ring[c] if et < 2 else qt_ring[c]
                    nc.vector.tensor_copy(
                        dst[:, et % 2, tb * 512 : (tb + 1) * 512], ps[:]
                    )

                cl.append(kq_group)
            for tt in range(4):  # V tok-tiles
                def v_group(c=c, tb=tb, tt=tt):
                    xt_t = xt_tiles[(c, tb)]
                    ps = ps_m.tile([128, 512], F32, tag="m")
                    for dt in range(8):
                        nc.tensor.matmul(
                            ps[:, 0:E],
                            xt_t[:, dt, tt * 128 : (tt + 1) * 128],
                            wv_sb[:, dt, :],
                            start=(dt == 0),
                            stop=(dt == 7),
                        )
                    nc.vector.tensor_copy(
                        v_ring[c][:, tb * 4 + tt, :, :],
                        ps[:, 0:E].rearrange("p (h d) -> p h d", h=HL),
                    )

                cl.append(v_group)
        return cl

    def outproj_closures(c):
        cl = []
        for tb in range(2):      # tb-major: tb=0 only needs qb=0 blocks
            for ddt in range(8):
                def op_group(c=c, ddt=ddt, tb=tb):
                    ot_c = ot_ring[c]
                    fp = ps_m.tile([128, 512], F32, tag="m")
                    for et in range(2):
                        nc.tensor.matmul(
                            fp[:],
                            wo_sb[:, et, ddt * 128 : (ddt + 1) * 128],
                            ot_c[:, et, tb * 512 : (tb + 1) * 512],
                            start=(et == 0),
                            stop=(et == 1),
                        )
                    fin = fpool.tile([128, 512], F16, tag="fin")
                    nc.vector.tensor_copy(fin[:], fp[:])
                    nc.sync.dma_start(
                        pt_d[
                            ddt * 128 : (ddt + 1) * 128,
                            c * CT + tb * 512 : c * CT + (tb + 1) * 512,
                        ],
                        fin[:],
                    )

                cl.append(op_group)
        return cl

    def sdpa_chunk(c, filler, tails):
        kts = (
            [(c, i) for i in range(8)]
            if c == 0
            else [(c - 1, i) for i in range(8)] + [(c, i) for i in range(8)]
        )
        ot_ring[c] = opool.tile([128, 2, CT], F16, tag="ot", name=f"ot{c}")
        ot_c = ot_ring[c]
        qt_c = qt_ring[c]
        n = len(kts)
        total_steps = 4 * n
        n_fill = len(filler)
        state = {"popped": 0, "step": 0}

        if stage == "proj":
            nc.vector.tensor_copy(ot_c[:, :, 0:4], kt_ring[c][:, :, 0:4])
            while tails:
                tails.pop(0)()
            while filler:
                filler.pop(0)()
            return

        def pace():
            state["step"] += 1
            if filler and state["popped"] * total_steps < state["step"] * n_fill:
                filler.pop(0)()
                state["popped"] += 1
            while tails:
                tails.pop(0)()
            while filler and state["popped"] * total_steps < state["step"] * n_fill:
                filler.pop(0)()
                state["popped"] += 1

        for hp in range(2):
            for qb in range(2):
                o_ps = ps_o.tile([128, 512], F32, tag="o")
                den_ps = ps_den.tile([128, 512], F32, tag="den")
                s_t, p_t = {}, {}

                def qk(i, hp=hp, qb=qb, s_t=s_t):
                    kc, kt = kts[i]
                    s = ps_s.tile([128, 1024], F32, tag="s")
                    ktile = kt_ring[kc]
                    for hl in range(2):
                        r0 = hl * 64
                        nc.tensor.matmul(
                            s[:, hl * 512 : hl * 512 + 512],
                            ktile[r0 : r0 + 64, hp, kt * 128 : (kt + 1) * 128],
                            qt_c[r0 : r0 + 64, hp, qb * 512 : (qb + 1) * 512],
                            start=True,
                            stop=True,
                            tile_position=(r0, 0),
                        )
                    s_t[i] = s

                qk(0)
                qk(1)
                last = n - 1
                for i in range(n):
                    kc, kt = kts[i]
                    p = ppool.tile([128, 1024], F16, tag="p")
                    if stage == "full":
                        nc.scalar.activation(p[:], s_t.pop(i)[:], EXP)
                    else:  # qkproj: skip exp, keep dep s->p
                        nc.vector.tensor_copy(p[:, 0:4], s_t.pop(i)[:, 0:4])
                    p_t[i] = p
                    pace()
                    if i + 2 < n:
                        qk(i + 2)
                    if stage != "full":
                        p_t.pop(i)
                        continue
                    vt = v_ring[kc]
                    nc.tensor.matmul(
                        o_ps[0:64, :], vt[:, kt, 2 * hp, :], p[:, 0:512],
                        start=(i == 0), stop=(i == last), tile_position=(0, 0),
                    )
                    nc.tensor.matmul(
                        o_ps[64:128, :], vt[:, kt, 2 * hp + 1, :], p[:, 512:1024],
                        start=(i == 0), stop=(i == last), tile_position=(0, 64),
                    )
                    if i % 2 == 1:
                        pe_, po_ = p_t.pop(i - 1), p_t.pop(i)
                        st, sp = (i == 1), (i == last)
                        nc.tensor.matmul(
                            den_ps[0:32, :], ones32, pe_[:, 0:512],
                            start=st, stop=sp, tile_position=(0, 0),
                        )
                        nc.tensor.matmul(
                            den_ps[32:64, :], ones32, pe_[:, 512:1024],
                            start=st, stop=sp, tile_position=(0, 32),
                        )
                        nc.tensor.matmul(
                            den_ps[64:96, :], ones32, po_[:, 0:512],
                            start=st, stop=sp, tile_position=(0, 64),
                        )
                        nc.tensor.matmul(
                            den_ps[96:128, :], ones32, po_[:, 512:1024],
                            start=st, stop=sp, tile_position=(0, 96),
                        )

                if stage != "full":
                    nc.vector.tensor_copy(ot_c[:, hp, qb * 512 : qb * 512 + 4],
                                          qt_c[:, hp, 0:4])
                    continue
                # block end: drain psum promptly (DVE), defer normalize
                osb = spool.tile([128, 512], F32, tag="osb")
                nc.vector.tensor_copy(osb[:], o_ps[:])
                den_sb = spool.tile([128, 512], F16, tag="dsb")
                nc.vector.tensor_copy(den_sb[:], den_ps[:])

                def tail(hp=hp, qb=qb, osb=osb, den_sb=den_sb):
                    bc_ps = ps_m.tile([128, 512], F32, tag="m")
                    nc.tensor.matmul(bc_ps[0:64, :], sel0, den_sb[:],
                                     start=True, stop=True, tile_position=(0, 0))
                    nc.tensor.matmul(bc_ps[64:128, :], sel1, den_sb[:],
                                     start=True, stop=True, tile_position=(0, 64))
                    bc_sb = spool.tile([128, 512], F32, tag="bc")
                    nc.vector.reciprocal(bc_sb[:], bc_ps[:])
                    nc.vector.tensor_mul(
                        ot_c[:, hp, qb * 512 : (qb + 1) * 512], osb[:], bc_sb[:]
                    )

                tails.append(tail)
        while filler:
            filler.pop(0)()

    # prologue: chunk 0 projections
    tails = []
    for cl in proj_closures(0):
        cl()
    for c in range(C):
        filler = []
        pj = proj_closures(c + 1) if c + 1 < C else []
        op = outproj_closures(c - 1) if c >= 1 else []
        while pj or op:
            if op:
                filler.append(op.pop(0))
            if pj:
                filler.append(pj.pop(0))
        sdpa_chunk(c, filler, tails)
    while tails:
        tails.pop(0)()
    for cl in outproj_closures(C - 1):
        cl()


def _build(reps: int = 1, stage: str = "full", hw_loop: int = 1):
    nc = bacc.Bacc(
        "TRN2",
        target_bir_lowering=False,
        debug=False,
        enable_asserts=False,
        num_devices=NCORES,
    )
    xt_d = nc.dram_tensor("xt", [D, T], F16, kind="ExternalInput").ap()
    wkq_d = nc.dram_tensor("wkq", [D, 2 * E], F16, kind="ExternalInput").ap()
    wv_d = nc.dram_tensor("wv", [D, E], F16, kind="ExternalInput").ap()
    wo_d = nc.dram_tensor("wo", [E, D], F16, kind="ExternalInput").ap()
    cst_d = nc.dram_tensor("cst", [128, 160], F16, kind="ExternalInput").ap()
    pt_d = nc.dram_tensor("pt", [D, T], F16, kind="ExternalOutput").ap()

    with tile.TileContext(nc) as tc:
        with (
            tc.tile_pool(name="const", bufs=1) as cpool,
            tc.tile_pool(name="xt", bufs=3) as xpool,
            tc.tile_pool(name="kt", bufs=4) as kpool,
            tc.tile_pool(name="qt", bufs=2) as qpool,
            tc.tile_pool(name="v", bufs=4) as vpool,
            tc.tile_pool(name="p", bufs=6) as ppool,
            tc.tile_pool(name="ot", bufs=2) as opool,
            tc.tile_pool(name="sm", bufs=8) as spool,
            tc.tile_pool(name="fin", bufs=4) as fpool,
            tc.tile_pool(name="ps_s", bufs=2, space="PSUM") as ps_s,
            tc.tile_pool(name="ps_o", bufs=1, space="PSUM") as ps_o,
            tc.tile_pool(name="ps_den", bufs=1, space="PSUM") as ps_den,
            tc.tile_pool(name="ps_m", bufs=2, space="PSUM") as ps_m,
        ):
            wkq_sb = cpool.tile([128, 8, 2 * E], F16, tag="wkq")
            nc.sync.dma_start(wkq_sb[:], wkq_d.rearrange("(dt p) e -> p dt e", p=128))
            wv_sb = cpool.tile([128, 8, E], F16, tag="wv")
            nc.sync.dma_start(wv_sb[:], wv_d.rearrange("(dt p) e -> p dt e", p=128))
            wo_sb = cpool.tile([128, 2, D], F16, tag="wo")
            nc.sync.dma_start(wo_sb[:], wo_d.rearrange("(et p) d -> p et d", p=128))
            cst_sb = cpool.tile([128, 160], F16, tag="cst")
            nc.sync.dma_start(cst_sb[:], cst_d)

            pools = (xpool, kpool, qpool, vpool, ppool, opool, spool, fpool,
                     ps_s, ps_o, ps_den, ps_m)
            aps = (xt_d, pt_d, wkq_sb, wv_sb, wo_sb, cst_sb)

            if hw_loop > 1:
                with tc.For_i(0, hw_loop, 1):
                    _emit_pass(nc, pools, aps, stage)
            else:
                for _ in range(reps):
                    _emit_pass(nc, pools, aps, stage)

    nc.compile()
    return nc


def _prepare_inputs(x, w_qkv, w_out):
    xt = []
    for b in range(B):
        xt.append(np.ascontiguousarray(x[b].reshape(T, D).T).astype(np.float16))
    cst = np.zeros((128, 160), dtype=np.float16)
    cst[:, 0:32] = 1.0
    cst[0, 32:96] = 1.0
    cst[64, 32:96] = 1.0
    cst[32, 96:160] = 1.0
    cst[96, 96:160] = 1.0
    in_maps = []
    for core in range(NCORES):
        b, hg = divmod(core, HL)
        e0 = hg * E
        wq = 0.125 * w_qkv[e0 : e0 + E]                 # fold 1/sqrt(HD)
        wk = w_qkv[H * HD + e0 : H * HD + e0 + E]
        wv = w_qkv[2 * H * HD + e0 : 2 * H * HD + e0 + E]
        wkq = np.ascontiguousarray(np.concatenate([wk, wq], axis=0).T).astype(np.float16)
        wv_t = np.ascontiguousarray(wv.T).astype(np.float16)
        wo_t = np.ascontiguousarray(w_out[:, e0 : e0 + E].T).astype(np.float16)
        in_maps.append(
            {"xt": xt[b], "wkq": wkq, "wv": wv_t, "wo": wo_t, "cst": cst}
        )
    return in_maps


def _gather(results):
    out = np.empty((B, F, N, D), dtype=np.float32)
    for b in range(B):
        acc = results[4 * b]["pt"].astype(np.float32)
        for hg in range(1, HL):
            acc += results[4 * b + hg]["pt"].astype(np.float32)
        out[b] = acc.T.reshape(F, N, D)
    return out


def run(x, w_qkv, w_out, trace=False, reps=1, stage="full", hw_loop=1):
    key = ("nc", reps, stage, hw_loop)
    if key not in _cached:
        _cached[key] = _build(reps, stage, hw_loop)
    nc = _cached[key]
    in_maps = _prepare_inputs(
        np.asarray(x, dtype=np.float32),
        np.asarray(w_qkv, dtype=np.float32),
        np.asarray(w_out, dtype=np.float32),
    )
    res = run_bass_kernel_spmd(nc, in_maps, core_ids=list(range(NCORES)), trace=trace)
    return _gather(res.results), res


def kernel(x, w_qkv, w_out):
    out, _ = run(x, w_qkv, w_out)
    return out


# revision 21
# speedup vs baseline: 1.1547x; 1.0425x over previous
"""LocalFrameAttention TRN2 kernel (v4).

Problem: x[B=2,F=16,N=256,D=1024] -> qkv proj -> chunked local attention
(chunk = 4 frames = 1024 tokens; chunk c attends to chunks {c-1, c}, chunk 0
to itself) -> out proj.  H=16 heads, HD=64.

Sharding: 8 cores = B(2) x head-groups(4).  Each core handles 4 heads for all
16 frames of one batch: column-parallel qkv projection, full SDPA for its
heads, row-parallel out projection producing a partial [D, T] fp16 output;
the host sums the 4 partials per batch and transposes.

Design notes (from HW microbenchmarks, micro.py):
- PE array-tiling concurrency is real: 64-contraction row pairs at array row
  groups 0/64 run concurrently (~101ns/MM), M=64 col pairs at col groups 0/64
  too (~118ns/MM).  BUT each tiling-MODE boundary in the instruction stream
  drains the PE (~115ns), and M=65 outputs (ones-column trick) are
  pathologically slow (~690ns/MM) -- avoided entirely.
- The exp pipeline (ACT, [128,1024] fp32 PSUM tiles, ~1.15us each, 224 of
  them) runs concurrently and is NOT the bottleneck; the PE is.  PSUM fits
  only 2 S tiles, locking QK issuance to the exp cadence, so matmuls are
  emitted in 2-step PERIODS with mode-class batching to minimize boundaries:
    per period: [PV pair x2 + sel tails (128x64)] [den quad (128x32)]
                [proj/outproj fillers (128x128)] [QK pair x2 (64x128,
                stretched across the period boundary)]
  ~4 boundaries/period instead of ~7.
- softmax denominators: 4-way col-tiled ones-matmuls (M=32 strips at col
  positions 0/32/64/96), one group per period; h0 partials land in strips
  0+64, h1 in 32+96; summed+broadcast by a col-tiled pair of selection-matrix
  matmuls (sel.T @ den -> 64 rows of 1/den after a DVE reciprocal).
- PSUM: S ping-pong 2x[128,1024] (4) + O [128,512] (1) + den (1) + misc 2 = 8.
- rotate mode (timing builds): chunk-0 projection and last-chunk
  out-projection become fillers of the adjacent hw-loop iteration, removing
  the inter-pass pipeline bubble.  Correctness builds (hw_loop=1) use the
  complete in-order schedule.

All matmul operands fp16 (fp32 PSUM).  pt output fp16.
"""

import sys

if "/opt/trn_rl_repo" not in sys.path:
    sys.path.insert(0, "/opt/trn_rl_repo")

import numpy as np

import concourse.bass as bass  # noqa: F401
import concourse.mybir as mybir
import concourse.tile as tile
from concourse import bacc
from concourse.bass_utils import run_bass_kernel_spmd

F32 = mybir.dt.float32
F16 = mybir.dt.float16
EXP = mybir.ActivationFunctionType.Exp

B, F, N, D = 2, 16, 256, 1024
H, HD, CHUNK = 16, 64, 4
C = F // CHUNK            # 4 chunks
CT = CHUNK * N            # 1024 tokens per chunk
T = F * N                 # 4096 tokens per batch
HL = 4                    # heads per core
E = HL * HD               # 256 local qkv width
NCORES = 8

_cached = {}


def _emit_pass(nc, pools, aps, stage, rotate=False):
    (xpool, kpool, qpool, vpool, ppool, opool, spool, fpool,
     ps_s, ps_o, ps_den, ps_m) = pools
    xt_d, pt_d, wkq_sb, wv_sb, wo_sb, cst_sb = aps

    ones32 = cst_sb[:, 0:32]     # all-ones [128,32] (denominator lhsT)
    sel0 = cst_sb[:, 32:96]      # ones at rows {0,64}  -> h_even denom bcast
    sel1 = cst_sb[:, 96:160]     # ones at rows {32,96} -> h_odd denom bcast

    kt_ring, qt_ring, v_ring, xt_tiles, ot_ring = {}, {}, {}, {}, {}

    do_qk = stage != "proj"
    do_exp = stage in ("qkexp", "full")
    do_pv = stage in ("noexp", "full")

    def alloc_kqv(c):
        kt_ring[c] = kpool.tile([128, 2, CT], F16, tag="kt", name=f"kt{c}")
        qt_ring[c] = qpool.tile([128, 2, CT], F16, tag="qt", name=f"qt{c}")
        v_ring[c] = vpool.tile([128, 8, HL, HD], F16, tag="v", name=f"v{c}")

    def proj_closures(c):
        """18 closures: 2 prefetch DMAs first, then 8 matmul groups per tb."""
        cl = []
        groups = []
        for tb in range(2):
            def dma(c=c, tb=tb):
                xt_t = xpool.tile([128, 8, 512], F16, tag="xt")
                t0 = c * CT + tb * 512
                nc.sync.dma_start(
                    xt_t[:],
                    xt_d[:, t0 : t0 + 512].rearrange("(dt p) t -> p dt t", p=128),
                )
                xt_tiles[(c, tb)] = xt_t

            cl.append(dma)
            for et in range(4):  # K^T (0,1), Q^T (2,3)
                def kq_group(c=c, tb=tb, et=et):
                    xt_t = xt_tiles[(c, tb)]
                    ps = ps_m.tile([128, 512], F32, tag="m")
                    for dt in range(8):
                        nc.tensor.matmul(
                            ps[:],
                            wkq_sb[:, dt, et * 128 : (et + 1) * 128],
                            xt_t[:, dt, :],
                            start=(dt == 0),
                            stop=(dt == 7),
                        )
                    dst = kt_ring[c] if et < 2 else qt_ring[c]
                    nc.vector.tensor_copy(
                        dst[:, et % 2, tb * 512 : (tb + 1) * 512], ps[:]
                    )

                groups.append(kq_group)
            for tt in range(4):  # V tok-tiles
                def v_group(c=c, tb=tb, tt=tt):
                    xt_t = xt_tiles[(c, tb)]
                    ps = ps_m.tile([128, 512], F32, tag="m")
                    for dt in range(8):
                        nc.tensor.matmul(
                            ps[:, 0:E],
                            xt_t[:, dt, tt * 128 : (tt + 1) * 128],
                            wv_sb[:, dt, :],
                            start=(dt == 0),
                            stop=(dt == 7),
                        )
                    nc.vector.tensor_copy(
                        v_ring[c][:, tb * 4 + tt, :, :],
                        ps[:, 0:E].rearrange("p (h d) -> p h d", h=HL),
                    )

                groups.append(v_group)
        cl.extend(groups)
        return cl

    def outproj_closures(c_out, ot_c):
        cl = []
        for tb in range(2):      # tb-major: tb=0 only needs qb=0 blocks
            for ddt in range(8):
                def op_group(c_out=c_out, ot_c=ot_c, ddt=ddt, tb=tb):
                    fp = ps_m.tile([128, 512], F32, tag="m")
                    for et in range(2):
                        nc.tensor.matmul(
                            fp[:],
                            wo_sb[:, et, ddt * 128 : (ddt + 1) * 128],
                            ot_c[:, et, tb * 512 : (tb + 1) * 512],
                            start=(et == 0),
                            stop=(et == 1),
                        )
                    fin = fpool.tile([128, 512], F16, tag="fin")
                    nc.scalar.copy(fin[:], fp[:])
                    nc.sync.dma_start(
                        pt_d[
                            ddt * 128 : (ddt + 1) * 128,
                            c_out * CT + tb * 512 : c_out * CT + (tb + 1) * 512,
                        ],
                        fin[:],
                    )

                cl.append(op_group)
        return cl

    def sdpa_chunk(c, filler, tails):
        kts = (
            [(c, i) for i in range(8)]
            if c == 0
            else [(c - 1, i) for i in range(8)] + [(c, i) for i in range(8)]
        )
        ot_ring[c] = opool.tile([128, 2, CT], F16, tag="ot", name=f"ot{c}")
        ot_c = ot_ring[c]
        qt_c = qt_ring[c]
        n = len(kts)
        total_steps = 4 * n
        n_fill = len(filler)
        state = {"popped": 0, "step": 0}

        if stage == "proj":
            nc.vector.tensor_copy(ot_c[:, :, 0:4], kt_ring[c][:, :, 0:4])
            while tails:
                tails.pop(0)()
            while filler:
                filler.pop(0)()
            return

        def pace(k=1):
            state["step"] += k
            while filler and state["popped"] * total_steps < state["step"] * n_fill:
                filler.pop(0)()
                state["popped"] += 1

        for hp in range(2):
            for qb in range(2):
                if do_pv:
                    o_ps = ps_o.tile([128, 512], F32, tag="o", name="o_ps")
                    den_ps = ps_den.tile([128, 512], F32, tag="den", name="den_ps")
                s_t, p_t = {}, {}

                def qk(i, hp=hp, qb=qb, s_t=s_t):
                    kc, kt = kts[i]
                    s = ps_s.tile([128, 1024], F32, tag="s")
                    ktile = kt_ring[kc]
                    for hl in range(2):
                        r0 = hl * 64
                        nc.tensor.matmul(
                            s[:, hl * 512 : hl * 512 + 512],
                            ktile[r0 : r0 + 64, hp, kt * 128 : (kt + 1) * 128],
                            qt_c[r0 : r0 + 64, hp, qb * 512 : (qb + 1) * 512],
                            start=True,
                            stop=True,
                            tile_position=(r0, 0),
                        )
                    s_t[i] = s

                def mk_p(i):
                    if do_exp:
                        p = ppool.tile([128, 1024], F16, tag="p")
                        nc.scalar.activation(p[:], s_t.pop(i)[:], EXP)
                        p_t[i] = p
                    elif do_pv:  # noexp: PV on near-unwritten p (timing only)
                        s_t.pop(i)
                        p_t[i] = ppool.tile([128, 1024], F16, tag="p", name="p")
                        nc.vector.memset(p_t[i][0:1, 0:4], 0.0)
                    else:        # qk: no consumer for s
                        s_t.pop(i)

                def pv_pair(i, last, hp=hp):
                    p = p_t[i]
                    kc, kt = kts[i]
                    vt = v_ring[kc]
                    nc.tensor.matmul(
                        o_ps[0:64, :], vt[:, kt, 2 * hp, :], p[:, 0:512],
                        start=(i == 0), stop=(i == last), tile_position=(0, 0),
                    )
                    nc.tensor.matmul(
                        o_ps[64:128, :], vt[:, kt, 2 * hp + 1, :], p[:, 512:1024],
                        start=(i == 0), stop=(i == last), tile_position=(0, 64),
                    )

                def den_quad(j, last):
                    pe_, po_ = p_t.pop(j), p_t.pop(j + 1)
                    st, sp = (j == 0), (j + 1 == last)
                    nc.tensor.matmul(den_ps[0:32, :], ones32, pe_[:, 0:512],
                                     start=st, stop=sp, tile_position=(0, 0))
                    nc.tensor.matmul(den_ps[32:64, :], ones32, pe_[:, 512:1024],
                                     start=st, stop=sp, tile_position=(0, 32))
                    nc.tensor.matmul(den_ps[64:96, :], ones32, po_[:, 0:512],
                                     start=st, stop=sp, tile_position=(0, 64))
                    nc.tensor.matmul(den_ps[96:128, :], ones32, po_[:, 512:1024],
                                     start=st, stop=sp, tile_position=(0, 96))

                # per-tick software pipeline (v2 order): exp(i) | fillers |
                # QK(i+2) | PV(i) | dens+tails every 2nd tick
                qk(0)
                qk(1)
                last = n - 1
                for i in range(n):
                    mk_p(i)
                    pace(1)
                    if i + 2 < n:
                        qk(i + 2)
                    if do_pv and i >= 1:
                        pv_pair(i - 1, last)
                        if i % 2 == 0 and i >= 2:
                            while tails:
                                tails.pop(0)()
                            den_quad(i - 2, last)
                if do_pv:
                    pv_pair(n - 1, last)
                    den_quad(n - 2, last)
                elif do_exp:
                    p_t.clear()

                if not do_pv:
                    nc.vector.tensor_copy(ot_c[:, hp, qb * 512 : qb * 512 + 4],
                                          qt_c[:, hp, 0:4])
                    continue
                # block end: drain psum promptly (DVE), defer normalize
                osb = spool.tile([128, 512], F32, tag="osb", name="osb")
                nc.vector.tensor_copy(osb[:], o_ps[:])
                den_sb = spool.tile([128, 512], F16, tag="dsb", name="dsb")
                nc.vector.tensor_copy(den_sb[:], den_ps[:])

                def tail(hp=hp, qb=qb, osb=osb, den_sb=den_sb):
                    bc_ps = ps_m.tile([128, 512], F32, tag="m", name="bc")
                    nc.tensor.matmul(bc_ps[0:64, :], sel0, den_sb[:],
                                     start=True, stop=True, tile_position=(0, 0))
                    nc.tensor.matmul(bc_ps[64:128, :], sel1, den_sb[:],
                                     start=True, stop=True, tile_position=(0, 64))
                    bc_sb = spool.tile([128, 512], F32, tag="bc", name="bc_sb")
                    nc.vector.reciprocal(bc_sb[:], bc_ps[:])
                    nc.vector.tensor_mul(
                        ot_c[:, hp, qb * 512 : (qb + 1) * 512], osb[:], bc_sb[:]
                    )

                tails.append(tail)
        while tails:
            tails.pop(0)()
        while filler:
            filler.pop(0)()

    def interleave(op, pj):
        filler = []
        while pj or op:
            if op:
                filler.append(op.pop(0))
            if pj:
                filler.append(pj.pop(0))
        return filler

    tails = []
    if rotate:
        # steady-state schedule for the hw timing loop: chunk-0 projection and
        # last-chunk out-projection are fillers of the adjacent iteration.
        alloc_kqv(0)  # carrier tiles written by the previous iteration
        ot_prev = opool.tile([128, 2, CT], F16, tag="ot", name="otp")
        # dummy writes so the allocator sees the carriers written (timing-only
        # builds; clobbers 4 columns of the previous iteration's data)
        nc.vector.memset(ot_prev[:, :, 0:4], 0.0)
        nc.vector.memset(kt_ring[0][:, :, 0:4], 0.0)
        nc.vector.memset(qt_ring[0][:, :, 0:4], 0.0)
        nc.vector.memset(v_ring[0][:, 0, :, 0:4], 0.0)
        for c in range(C):
            nxt = (c + 1) % C
            alloc_kqv(nxt)
            op = outproj_closures(C - 1, ot_prev) if c == 0 else \
                outproj_closures(c - 1, ot_ring[c - 1])
            sdpa_chunk(c, interleave(op, proj_closures(nxt)), tails)
        while tails:
            tails.pop(0)()
    else:
        alloc_kqv(0)
        for cl in proj_closures(0):
            cl()
        for c in range(C):
            if c + 1 < C:
                alloc_kqv(c + 1)
            pj = proj_closures(c + 1) if c + 1 < C else []
            op = outproj_closures(c - 1, ot_ring[c - 1]) if c >= 1 else []
            sdpa_chunk(c, interleave(op, pj), tails)
        while tails:
            tails.pop(0)()
        for cl in outproj_closures(C - 1, ot_ring[C - 1]):
            cl()


def _build(reps: int = 1, stage: str = "full", hw_loop: int = 1,
           rotate: bool = False):
    nc = bacc.Bacc(
        "TRN2",
        target_bir_lowering=False,
        debug=False,
        enable_asserts=False,
        num_devices=NCORES,
    )
    xt_d = nc.dram_tensor("xt", [D, T], F16, kind="ExternalInput").ap()
    wkq_d = nc.dram_tensor("wkq", [D, 2 * E], F16, kind="ExternalInput").ap()
    wv_d = nc.dram_tensor("wv", [D, E], F16, kind="ExternalInput").ap()
    wo_d = nc.dram_tensor("wo", [E, D], F16, kind="ExternalInput").ap()
    cst_d = nc.dram_tensor("cst", [128, 160], F16, kind="ExternalInput").ap()
    pt_d = nc.dram_tensor("pt", [D, T], F16, kind="ExternalOutput").ap()

    with tile.TileContext(nc) as tc:
        with (
            tc.tile_pool(name="const", bufs=1) as cpool,
            tc.tile_pool(name="xt", bufs=4) as xpool,
            tc.tile_pool(name="kt", bufs=4) as kpool,
            tc.tile_pool(name="qt", bufs=2) as qpool,
            tc.tile_pool(name="v", bufs=4) as vpool,
            tc.tile_pool(name="p", bufs=8) as ppool,
            tc.tile_pool(name="ot", bufs=2) as opool,
            tc.tile_pool(name="sm", bufs=8) as spool,
            tc.tile_pool(name="fin", bufs=6) as fpool,
            tc.tile_pool(name="ps_s", bufs=2, space="PSUM") as ps_s,
            tc.tile_pool(name="ps_o", bufs=1, space="PSUM") as ps_o,
            tc.tile_pool(name="ps_den", bufs=1, space="PSUM") as ps_den,
            tc.tile_pool(name="ps_m", bufs=2, space="PSUM") as ps_m,
        ):
            wkq_sb = cpool.tile([128, 8, 2 * E], F16, tag="wkq")
            nc.sync.dma_start(wkq_sb[:], wkq_d.rearrange("(dt p) e -> p dt e", p=128))
            wv_sb = cpool.tile([128, 8, E], F16, tag="wv")
            nc.sync.dma_start(wv_sb[:], wv_d.rearrange("(dt p) e -> p dt e", p=128))
            wo_sb = cpool.tile([128, 2, D], F16, tag="wo")
            nc.sync.dma_start(wo_sb[:], wo_d.rearrange("(et p) d -> p et d", p=128))
            cst_sb = cpool.tile([128, 160], F16, tag="cst")
            nc.sync.dma_start(cst_sb[:], cst_d)

            pools = (xpool, kpool, qpool, vpool, ppool, opool, spool, fpool,
                     ps_s, ps_o, ps_den, ps_m)
            aps = (xt_d, pt_d, wkq_sb, wv_sb, wo_sb, cst_sb)

            if hw_loop > 1:
                with tc.For_i(0, hw_loop, 1):
                    _emit_pass(nc, pools, aps, stage, rotate=rotate)
            else:
                for _ in range(reps):
                    _emit_pass(nc, pools, aps, stage, rotate=rotate)

    nc.compile()
    return nc


def _prepare_inputs(x, w_qkv, w_out):
    xt = []
    for b in range(B):
        xt.append(np.ascontiguousarray(x[b].reshape(T, D).T).astype(np.float16))
    cst = np.zeros((128, 160), dtype=np.float16)
    cst[:, 0:32] = 1.0
    cst[0, 32:96] = 1.0
    cst[64, 32:96] = 1.0
    cst[32, 96:160] = 1.0
    cst[96, 96:160] = 1.0
    in_maps = []
    for core in range(NCORES):
        b, hg = divmod(core, HL)
        e0 = hg * E
        wq = 0.125 * w_qkv[e0 : e0 + E]                 # fold 1/sqrt(HD)
        wk = w_qkv[H * HD + e0 : H * HD + e0 + E]
        wv = w_qkv[2 * H * HD + e0 : 2 * H * HD + e0 + E]
        wkq = np.ascontiguousarray(np.concatenate([wk, wq], axis=0).T).astype(np.float16)
        wv_t = np.ascontiguousarray(wv.T).astype(np.float16)
        wo_t = np.ascontiguousarray(w_out[:, e0 : e0 + E].T).astype(np.float16)
        in_maps.append(
            {"xt": xt[b], "wkq": wkq, "wv": wv_t, "wo": wo_t, "cst": cst}
        )
    return in_maps


def _gather(results):
    out = np.empty((B, F, N, D), dtype=np.float32)
    for b in range(B):
        acc = results[4 * b]["pt"].astype(np.float32)
        for hg in range(1, HL):
            acc += results[4 * b + hg]["pt"].astype(np.float32)
        out[b] = acc.T.reshape(F, N, D)
    return out


def run(x, w_qkv, w_out, trace=False, reps=1, stage="full", hw_loop=1,
        rotate=False):
    key = ("nc", reps, stage, hw_loop, rotate)
    if key not in _cached:
        _cached[key] = _build(reps, stage, hw_loop, rotate)
    nc = _cached[key]
    in_maps = _prepare_inputs(
        np.asarray(x, dtype=np.float32),
        np.asarray(w_qkv, dtype=np.float32),
        np.asarray(w_out, dtype=np.float32),
    )
    res = run_bass_kernel_spmd(nc, in_maps, core_ids=list(range(NCORES)), trace=trace)
    return _gather(res.results), res


def kernel(x, w_qkv, w_out):
    out, _ = run(x, w_qkv, w_out)
    return out


# revision 22
# speedup vs baseline: 1.1743x; 1.0170x over previous
"""LocalFrameAttention TRN2 kernel (v4).

Problem: x[B=2,F=16,N=256,D=1024] -> qkv proj -> chunked local attention
(chunk = 4 frames = 1024 tokens; chunk c attends to chunks {c-1, c}, chunk 0
to itself) -> out proj.  H=16 heads, HD=64.

Sharding: 8 cores = B(2) x head-groups(4).  Each core handles 4 heads for all
16 frames of one batch: column-parallel qkv projection, full SDPA for its
heads, row-parallel out projection producing a partial [D, T] fp16 output;
the host sums the 4 partials per batch and transposes.

Design notes (from HW microbenchmarks, micro.py):
- PE array-tiling concurrency is real: 64-contraction row pairs at array row
  groups 0/64 run concurrently (~101ns/MM), M=64 col pairs at col groups 0/64
  too (~118ns/MM).  BUT each tiling-MODE boundary in the instruction stream
  drains the PE (~115ns), and M=65 outputs (ones-column trick) are
  pathologically slow (~690ns/MM) -- avoided entirely.
- The exp pipeline (ACT, [128,1024] fp32 PSUM tiles, ~1.15us each, 224 of
  them) runs concurrently and is NOT the bottleneck; the PE is.  PSUM fits
  only 2 S tiles, locking QK issuance to the exp cadence, so matmuls are
  emitted in 2-step PERIODS with mode-class batching to minimize boundaries:
    per period: [PV pair x2 + sel tails (128x64)] [den quad (128x32)]
                [proj/outproj fillers (128x128)] [QK pair x2 (64x128,
                stretched across the period boundary)]
  ~4 boundaries/period instead of ~7.
- softmax denominators: 4-way col-tiled ones-matmuls (M=32 strips at col
  positions 0/32/64/96), one group per period; h0 partials land in strips
  0+64, h1 in 32+96; summed+broadcast by a col-tiled pair of selection-matrix
  matmuls (sel.T @ den -> 64 rows of 1/den after a DVE reciprocal).
- PSUM: S ping-pong 2x[128,1024] (4) + O [128,512] (1) + den (1) + misc 2 = 8.
- rotate mode (timing builds): chunk-0 projection and last-chunk
  out-projection become fillers of the adjacent hw-loop iteration, removing
  the inter-pass pipeline bubble.  Correctness builds (hw_loop=1) use the
  complete in-order schedule.

All matmul operands fp16 (fp32 PSUM).  pt output fp16.
"""

import sys

if "/opt/trn_rl_repo" not in sys.path:
    sys.path.insert(0, "/opt/trn_rl_repo")

import numpy as np

import concourse.bass as bass  # noqa: F401
import concourse.mybir as mybir
import concourse.tile as tile
from concourse import bacc
from concourse.bass_utils import run_bass_kernel_spmd

F32 = mybir.dt.float32
F16 = mybir.dt.float16
EXP = mybir.ActivationFunctionType.Exp

B, F, N, D = 2, 16, 256, 1024
H, HD, CHUNK = 16, 64, 4
C = F // CHUNK            # 4 chunks
CT = CHUNK * N            # 1024 tokens per chunk
T = F * N                 # 4096 tokens per batch
HL = 4                    # heads per core
E = HL * HD               # 256 local qkv width
NCORES = 8

_cached = {}


def _emit_pass(nc, pools, aps, stage, rotate=False):
    (xpool, kpool, qpool, vpool, ppool, opool, spool, fpool,
     ps_s, ps_o, ps_den, ps_m) = pools
    xt_d, pt_d, wkq_sb, wv_sb, wo_sb, cst_sb = aps

    ones32 = cst_sb[:, 0:32]     # all-ones [128,32] (denominator lhsT)
    sel0 = cst_sb[:, 32:96]      # ones at rows {0,64}  -> h_even denom bcast
    sel1 = cst_sb[:, 96:160]     # ones at rows {32,96} -> h_odd denom bcast

    kt_ring, qt_ring, v_ring, xt_tiles, ot_ring = {}, {}, {}, {}, {}

    do_qk = stage != "proj"
    do_exp = stage in ("qkexp", "full")
    do_pv = stage in ("noexp", "full")

    def alloc_kqv(c):
        kt_ring[c] = kpool.tile([128, 2, CT], F16, tag="kt", name=f"kt{c}")
        qt_ring[c] = qpool.tile([128, 2, CT], F16, tag="qt", name=f"qt{c}")
        v_ring[c] = vpool.tile([128, 8, HL, HD], F16, tag="v", name=f"v{c}")

    def proj_closures(c):
        """18 closures: 2 prefetch DMAs first, then 8 matmul groups per tb."""
        cl = []
        groups = []
        for tb in range(2):
            def dma(c=c, tb=tb):
                xt_t = xpool.tile([128, 8, 512], F16, tag="xt")
                t0 = c * CT + tb * 512
                nc.sync.dma_start(
                    xt_t[:],
                    xt_d[:, t0 : t0 + 512].rearrange("(dt p) t -> p dt t", p=128),
                )
                xt_tiles[(c, tb)] = xt_t

            cl.append(dma)
            for et in range(4):  # K^T (0,1), Q^T (2,3)
                def kq_group(c=c, tb=tb, et=et):
                    xt_t = xt_tiles[(c, tb)]
                    ps = ps_m.tile([128, 512], F32, tag="m")
                    for dt in range(8):
                        nc.tensor.matmul(
                            ps[:],
                            wkq_sb[:, dt, et * 128 : (et + 1) * 128],
                            xt_t[:, dt, :],
                            start=(dt == 0),
                            stop=(dt == 7),
                        )
                    dst = kt_ring[c] if et < 2 else qt_ring[c]
                    nc.vector.tensor_copy(
                        dst[:, et % 2, tb * 512 : (tb + 1) * 512], ps[:]
                    )

                groups.append(kq_group)
            for tt in range(4):  # V tok-tiles
                def v_group(c=c, tb=tb, tt=tt):
                    xt_t = xt_tiles[(c, tb)]
                    ps = ps_m.tile([128, 512], F32, tag="m")
                    for dt in range(8):
                        nc.tensor.matmul(
                            ps[:, 0:E],
                            xt_t[:, dt, tt * 128 : (tt + 1) * 128],
                            wv_sb[:, dt, :],
                            start=(dt == 0),
                            stop=(dt == 7),
                        )
                    nc.vector.tensor_copy(
                        v_ring[c][:, tb * 4 + tt, :, :],
                        ps[:, 0:E].rearrange("p (h d) -> p h d", h=HL),
                    )

                groups.append(v_group)
        cl.extend(groups)
        return cl

    def outproj_closures(c_out, ot_c):
        cl = []
        for tb in range(2):      # tb-major: tb=0 only needs qb=0 blocks
            for ddt in range(8):
                def op_group(c_out=c_out, ot_c=ot_c, ddt=ddt, tb=tb):
                    fp = ps_m.tile([128, 512], F32, tag="m")
                    for et in range(2):
                        nc.tensor.matmul(
                            fp[:],
                            wo_sb[:, et, ddt * 128 : (ddt + 1) * 128],
                            ot_c[:, et, tb * 512 : (tb + 1) * 512],
                            start=(et == 0),
                            stop=(et == 1),
                        )
                    fin = fpool.tile([128, 512], F16, tag="fin")
                    nc.vector.tensor_copy(fin[:], fp[:])
                    nc.sync.dma_start(
                        pt_d[
                            ddt * 128 : (ddt + 1) * 128,
                            c_out * CT + tb * 512 : c_out * CT + (tb + 1) * 512,
                        ],
                        fin[:],
                    )

                cl.append(op_group)
        return cl

    def sdpa_chunk(c, filler, tails):
        kts = (
            [(c, i) for i in range(8)]
            if c == 0
            else [(c - 1, i) for i in range(8)] + [(c, i) for i in range(8)]
        )
        ot_ring[c] = opool.tile([128, 2, CT], F16, tag="ot", name=f"ot{c}")
        ot_c = ot_ring[c]
        qt_c = qt_ring[c]
        n = len(kts)
        total_steps = 4 * n
        n_fill = len(filler)
        state = {"popped": 0, "step": 0}

        if stage == "proj":
            nc.vector.tensor_copy(ot_c[:, :, 0:4], kt_ring[c][:, :, 0:4])
            while tails:
                tails.pop(0)()
            while filler:
                filler.pop(0)()
            return

        def pace(k=1):
            state["step"] += k
            while filler and state["popped"] * total_steps < state["step"] * n_fill:
                filler.pop(0)()
                state["popped"] += 1

        for hp in range(2):
            for qb in range(2):
                if do_pv:
                    o_ps = ps_o.tile([128, 512], F32, tag="o", name="o_ps")
                    den_ps = ps_den.tile([128, 512], F32, tag="den", name="den_ps")
                s_t, p_t = {}, {}

                def qk(i, hp=hp, qb=qb, s_t=s_t):
                    kc, kt = kts[i]
                    s = ps_s.tile([128, 1024], F32, tag="s")
                    ktile = kt_ring[kc]
                    for hl in range(2):
                        r0 = hl * 64
                        nc.tensor.matmul(
                            s[:, hl * 512 : hl * 512 + 512],
                            ktile[r0 : r0 + 64, hp, kt * 128 : (kt + 1) * 128],
                            qt_c[r0 : r0 + 64, hp, qb * 512 : (qb + 1) * 512],
                            start=True,
                            stop=True,
                            tile_position=(r0, 0),
                        )
                    s_t[i] = s

                def mk_p(i):
                    if do_exp:
                        p = ppool.tile([128, 1024], F16, tag="p")
                        nc.scalar.activation(p[:], s_t.pop(i)[:], EXP)
                        p_t[i] = p
                    elif do_pv:  # noexp: PV on near-unwritten p (timing only)
                        s_t.pop(i)
                        p_t[i] = ppool.tile([128, 1024], F16, tag="p", name="p")
                        nc.vector.memset(p_t[i][0:1, 0:4], 0.0)
                    else:        # qk: no consumer for s
                        s_t.pop(i)

                def pv_pair(i, last, hp=hp):
                    p = p_t[i]
                    kc, kt = kts[i]
                    vt = v_ring[kc]
                    nc.tensor.matmul(
                        o_ps[0:64, :], vt[:, kt, 2 * hp, :], p[:, 0:512],
                        start=(i == 0), stop=(i == last), tile_position=(0, 0),
                    )
                    nc.tensor.matmul(
                        o_ps[64:128, :], vt[:, kt, 2 * hp + 1, :], p[:, 512:1024],
                        start=(i == 0), stop=(i == last), tile_position=(0, 64),
                    )

                def den_quad(j, last):
                    pe_, po_ = p_t.pop(j), p_t.pop(j + 1)
                    st, sp = (j == 0), (j + 1 == last)
                    nc.tensor.matmul(den_ps[0:32, :], ones32, pe_[:, 0:512],
                                     start=st, stop=sp, tile_position=(0, 0))
                    nc.tensor.matmul(den_ps[32:64, :], ones32, pe_[:, 512:1024],
                                     start=st, stop=sp, tile_position=(0, 32))
                    nc.tensor.matmul(den_ps[64:96, :], ones32, po_[:, 0:512],
                                     start=st, stop=sp, tile_position=(0, 64))
                    nc.tensor.matmul(den_ps[96:128, :], ones32, po_[:, 512:1024],
                                     start=st, stop=sp, tile_position=(0, 96))

                # per-tick software pipeline (v2 order): exp(i) | fillers |
                # QK(i+2) | PV(i) | dens+tails every 2nd tick
                qk(0)
                qk(1)
                last = n - 1
                for i in range(n):
                    mk_p(i)
                    pace(1)
                    if i + 2 < n:
                        qk(i + 2)
                    if do_pv and i >= 1:
                        pv_pair(i - 1, last)
                        if i % 2 == 0 and i >= 2:
                            while tails:
                                tails.pop(0)()
                            den_quad(i - 2, last)
                if do_pv:
                    pv_pair(n - 1, last)
                    den_quad(n - 2, last)
                elif do_exp:
                    p_t.clear()

                if not do_pv:
                    nc.vector.tensor_copy(ot_c[:, hp, qb * 512 : qb * 512 + 4],
                                          qt_c[:, hp, 0:4])
                    continue
                # block end: drain psum promptly (DVE), defer normalize
                osb = spool.tile([128, 512], F32, tag="osb", name="osb")
                nc.vector.tensor_copy(osb[:], o_ps[:])
                den_sb = spool.tile([128, 512], F16, tag="dsb", name="dsb")
                nc.vector.tensor_copy(den_sb[:], den_ps[:])

                def tail(hp=hp, qb=qb, osb=osb, den_sb=den_sb):
                    bc_ps = ps_m.tile([128, 512], F32, tag="m", name="bc")
                    nc.tensor.matmul(bc_ps[0:64, :], sel0, den_sb[:],
                                     start=True, stop=True, tile_position=(0, 0))
                    nc.tensor.matmul(bc_ps[64:128, :], sel1, den_sb[:],
                                     start=True, stop=True, tile_position=(0, 64))
                    bc_sb = spool.tile([128, 512], F32, tag="bc", name="bc_sb")
                    nc.vector.reciprocal(bc_sb[:], bc_ps[:])
                    nc.vector.tensor_mul(
                        ot_c[:, hp, qb * 512 : (qb + 1) * 512], osb[:], bc_sb[:]
                    )

                tails.append(tail)
        while tails:
            tails.pop(0)()
        while filler:
            filler.pop(0)()

    def interleave(op, pj):
        filler = []
        while pj or op:
            if op:
                filler.append(op.pop(0))
            if pj:
                filler.append(pj.pop(0))
        return filler

    tails = []
    if rotate:
        # steady-state schedule for the hw timing loop: chunk-0 projection and
        # last-chunk out-projection are fillers of the adjacent iteration.
        alloc_kqv(0)  # carrier tiles written by the previous iteration
        ot_prev = opool.tile([128, 2, CT], F16, tag="ot", name="otp")
        # dummy writes so the allocator sees the carriers written (timing-only
        # builds; clobbers 4 columns of the previous iteration's data)
        nc.vector.memset(ot_prev[:, :, 0:4], 0.0)
        nc.vector.memset(kt_ring[0][:, :, 0:4], 0.0)
        nc.vector.memset(qt_ring[0][:, :, 0:4], 0.0)
        nc.vector.memset(v_ring[0][:, 0, :, 0:4], 0.0)
        for c in range(C):
            nxt = (c + 1) % C
            alloc_kqv(nxt)
            op = outproj_closures(C - 1, ot_prev) if c == 0 else \
                outproj_closures(c - 1, ot_ring[c - 1])
            sdpa_chunk(c, interleave(op, proj_closures(nxt)), tails)
        while tails:
            tails.pop(0)()
    else:
        alloc_kqv(0)
        for cl in proj_closures(0):
            cl()
        for c in range(C):
            if c + 1 < C:
                alloc_kqv(c + 1)
            pj = proj_closures(c + 1) if c + 1 < C else []
            op = outproj_closures(c - 1, ot_ring[c - 1]) if c >= 1 else []
            sdpa_chunk(c, interleave(op, pj), tails)
        while tails:
            tails.pop(0)()
        for cl in outproj_closures(C - 1, ot_ring[C - 1]):
            cl()


def _build(reps: int = 1, stage: str = "full", hw_loop: int = 1,
           rotate: bool = False):
    nc = bacc.Bacc(
        "TRN2",
        target_bir_lowering=False,
        debug=False,
        enable_asserts=False,
        num_devices=NCORES,
    )
    xt_d = nc.dram_tensor("xt", [D, T], F16, kind="ExternalInput").ap()
    wkq_d = nc.dram_tensor("wkq", [D, 2 * E], F16, kind="ExternalInput").ap()
    wv_d = nc.dram_tensor("wv", [D, E], F16, kind="ExternalInput").ap()
    wo_d = nc.dram_tensor("wo", [E, D], F16, kind="ExternalInput").ap()
    cst_d = nc.dram_tensor("cst", [128, 160], F16, kind="ExternalInput").ap()
    pt_d = nc.dram_tensor("pt", [D, T], F16, kind="ExternalOutput").ap()

    with tile.TileContext(nc) as tc:
        with (
            tc.tile_pool(name="const", bufs=1) as cpool,
            tc.tile_pool(name="xt", bufs=3) as xpool,
            tc.tile_pool(name="kt", bufs=4) as kpool,
            tc.tile_pool(name="qt", bufs=2) as qpool,
            tc.tile_pool(name="v", bufs=4) as vpool,
            tc.tile_pool(name="p", bufs=6) as ppool,
            tc.tile_pool(name="ot", bufs=2) as opool,
            tc.tile_pool(name="sm", bufs=8) as spool,
            tc.tile_pool(name="fin", bufs=4) as fpool,
            tc.tile_pool(name="ps_s", bufs=2, space="PSUM") as ps_s,
            tc.tile_pool(name="ps_o", bufs=1, space="PSUM") as ps_o,
            tc.tile_pool(name="ps_den", bufs=1, space="PSUM") as ps_den,
            tc.tile_pool(name="ps_m", bufs=2, space="PSUM") as ps_m,
        ):
            wkq_sb = cpool.tile([128, 8, 2 * E], F16, tag="wkq")
            nc.sync.dma_start(wkq_sb[:], wkq_d.rearrange("(dt p) e -> p dt e", p=128))
            wv_sb = cpool.tile([128, 8, E], F16, tag="wv")
            nc.sync.dma_start(wv_sb[:], wv_d.rearrange("(dt p) e -> p dt e", p=128))
            wo_sb = cpool.tile([128, 2, D], F16, tag="wo")
            nc.sync.dma_start(wo_sb[:], wo_d.rearrange("(et p) d -> p et d", p=128))
            cst_sb = cpool.tile([128, 160], F16, tag="cst")
            nc.sync.dma_start(cst_sb[:], cst_d)

            pools = (xpool, kpool, qpool, vpool, ppool, opool, spool, fpool,
                     ps_s, ps_o, ps_den, ps_m)
            aps = (xt_d, pt_d, wkq_sb, wv_sb, wo_sb, cst_sb)

            if hw_loop > 1:
                with tc.For_i(0, hw_loop, 1):
                    _emit_pass(nc, pools, aps, stage, rotate=rotate)
            else:
                for _ in range(reps):
                    _emit_pass(nc, pools, aps, stage, rotate=rotate)

    nc.compile()
    return nc


def _prepare_inputs(x, w_qkv, w_out):
    xt = []
    for b in range(B):
        xt.append(np.ascontiguousarray(x[b].reshape(T, D).T).astype(np.float16))
    cst = np.zeros((128, 160), dtype=np.float16)
    cst[:, 0:32] = 1.0
    cst[0, 32:96] = 1.0
    cst[64, 32:96] = 1.0
    cst[32, 96:160] = 1.0
    cst[96, 96:160] = 1.0
    in_maps = []
    for core in range(NCORES):
        b, hg = divmod(core, HL)
        e0 = hg * E
        wq = 0.125 * w_qkv[e0 : e0 + E]                 # fold 1/sqrt(HD)
        wk = w_qkv[H * HD + e0 : H * HD + e0 + E]
        wv = w_qkv[2 * H * HD + e0 : 2 * H * HD + e0 + E]
        wkq = np.ascontiguousarray(np.concatenate([wk, wq], axis=0).T).astype(np.float16)
        wv_t = np.ascontiguousarray(wv.T).astype(np.float16)
        wo_t = np.ascontiguousarray(w_out[:, e0 : e0 + E].T).astype(np.float16)
        in_maps.append(
            {"xt": xt[b], "wkq": wkq, "wv": wv_t, "wo": wo_t, "cst": cst}
        )
    return in_maps


def _gather(results):
    out = np.empty((B, F, N, D), dtype=np.float32)
    for b in range(B):
        acc = results[4 * b]["pt"].astype(np.float32)
        for hg in range(1, HL):
            acc += results[4 * b + hg]["pt"].astype(np.float32)
        out[b] = acc.T.reshape(F, N, D)
    return out


def run(x, w_qkv, w_out, trace=False, reps=1, stage="full", hw_loop=1,
        rotate=False):
    key = ("nc", reps, stage, hw_loop, rotate)
    if key not in _cached:
        _cached[key] = _build(reps, stage, hw_loop, rotate)
    nc = _cached[key]
    in_maps = _prepare_inputs(
        np.asarray(x, dtype=np.float32),
        np.asarray(w_qkv, dtype=np.float32),
        np.asarray(w_out, dtype=np.float32),
    )
    res = run_bass_kernel_spmd(nc, in_maps, core_ids=list(range(NCORES)), trace=trace)
    return _gather(res.results), res


def kernel(x, w_qkv, w_out):
    out, _ = run(x, w_qkv, w_out)
    return out
